# revision 9
# baseline (speedup 1.0000x reference)
"""Trainium2 Bass kernel for multi-head attention with RoPE (B=2, S=2048,
D=2048, H=16), distributed over 8 NeuronCores with head tensor-parallelism
and an AllToAll to switch to token-parallelism for the output projection.

kernel(**inputs) takes the full unsharded inputs (as produced by the
reference setup_inputs) and returns the full [2, 2048, 2048] f32 output.
"""
import numpy as np
import ml_dtypes
from concourse import bass, bacc, tile, mybir
from concourse.bass_utils import run_bass_kernel_spmd

bf16 = ml_dtypes.bfloat16
BF16 = mybir.dt.bfloat16
F32 = mybir.dt.float32
AF = mybir.ActivationFunctionType
OP = mybir.AluOpType

B, S, D, H = 2, 2048, 2048, 16
HD = 128                 # head dim
NCORES = 8
HL = H // NCORES         # heads per core = 2
EL = HL * HD             # local projection width = 256
T = B * S                # 4096 flattened tokens
TCH = 512                # token chunk for QKV phase
NTCH = T // TCH          # 8
NKT = S // 128           # 16 key tiles per batch
NQC = S // 512           # 4 query chunks per batch
NDT = D // 128           # 16 contraction tiles
TL = T // NCORES         # 512 tokens per core after AllToAll
SCALE = float(1.0 / np.sqrt(128.0))

_CACHE = {}
_DEBUG = False


def _build():
    nc = bacc.Bacc("TRN2", target_bir_lowering=False, num_devices=NCORES)
    dbg = {}
    if _DEBUG:
        dbg["qT0"] = nc.dram_tensor("dbg_qT0", [128, T], BF16, kind="ExternalOutput")
        dbg["kT0"] = nc.dram_tensor("dbg_kT0", [128, T], BF16, kind="ExternalOutput")
        dbg["vn0"] = nc.dram_tensor("dbg_vn0", [128, T], BF16, kind="ExternalOutput")
        dbg["xt0"] = nc.dram_tensor("dbg_xt0", [128, TCH], BF16, kind="ExternalOutput")
        dbg["E0"] = nc.dram_tensor("dbg_E0", [128, S], BF16, kind="ExternalOutput")
        dbg["on0"] = nc.dram_tensor("dbg_on0", [128, 512], BF16, kind="ExternalOutput")
        dbg["rec0"] = nc.dram_tensor("dbg_rec0", [128, 512], F32, kind="ExternalOutput")
        dbg["ot0"] = nc.dram_tensor("dbg_ot0", [128, TL], BF16, kind="ExternalOutput")
        dbg["a2ain"] = nc.dram_tensor("dbg_a2ain", [NCORES, EL, TL], BF16, kind="ExternalOutput")
        dbg["a2aout"] = nc.dram_tensor("dbg_a2aout", [NCORES, EL, TL], BF16, kind="ExternalOutput")
        dbg["y0"] = nc.dram_tensor("dbg_y0", [128, 512], F32, kind="ExternalOutput")

    x = nc.dram_tensor("x", [T, D], F32, kind="ExternalInput")
    wq_t = nc.dram_tensor("wq_t", [D, EL], BF16, kind="ExternalInput")
    wk_t = nc.dram_tensor("wk_t", [D, EL], BF16, kind="ExternalInput")
    wv_t = nc.dram_tensor("wv_t", [D, EL], BF16, kind="ExternalInput")
    wo_t = nc.dram_tensor("wo_t", [D, D], BF16, kind="ExternalInput")
    cos_t = nc.dram_tensor("cos_t", [HD, S], F32, kind="ExternalInput")
    sin_m = nc.dram_tensor("sin_m", [HD, S], F32, kind="ExternalInput")
    mask_t = nc.dram_tensor("mask_t", [128, B * NKT], F32, kind="ExternalInput")
    out = nc.dram_tensor("out", [TL, D], F32, kind="ExternalOutput")

    ident_dram = nc.inline_tensor(np.eye(128, dtype=bf16), name="ident")
    ones_dram = nc.inline_tensor(np.ones((128, 128), dtype=bf16), name="ones")

    with tile.TileContext(nc) as tc:
        with (
            # ---------- persistent pools (whole kernel) ----------
            tc.tile_pool(name="dram", bufs=1, space="DRAM") as dram,
            tc.tile_pool(name="consts", bufs=1) as consts,
            tc.tile_pool(name="qkv_keep", bufs=1) as keep,
        ):
            xbf = [dram.tile([TCH, D], BF16, tag=f"xbf{c}", name=f"xbf{c}")
                   for c in range(NTCH)]
            a2a_in = dram.tile([NCORES, EL, TL], BF16, tag="a2a_in", name="a2a_in")
            a2a_out = dram.tile([NCORES, EL, TL], BF16, tag="a2a_out", name="a2a_out")

            ident_sb = consts.tile([128, 128], BF16, tag="ident", name="ident_sb")
            nc.sync.dma_start(ident_sb[:], ident_dram[:])
            ones_sb = consts.tile([128, 128], BF16, tag="ones", name="ones_sb")
            nc.sync.dma_start(ones_sb[:], ones_dram[:])
            mask_sb = consts.tile([128, B * NKT], F32, tag="mask", name="mask_sb")
            nc.sync.dma_start(mask_sb[:], mask_t[:])

            # persistent per-head tensors ([hd, t] layouts; v natural packed
            # as 32 [t=128, hd=128] tiles along free dim)
            qT = [keep.tile([128, T], BF16, tag=f"qT{h}", name=f"qT{h}") for h in range(HL)]
            kT = [keep.tile([128, T], BF16, tag=f"kT{h}", name=f"kT{h}") for h in range(HL)]
            vnat = [keep.tile([128, T], BF16, tag=f"vn{h}", name=f"vn{h}") for h in range(HL)]

            # ---------- phase A+B: x prep + QKV projections + RoPE ----------
            with (
                tc.tile_pool(name="wsb", bufs=1) as wpool,
                tc.tile_pool(name="tables", bufs=1) as tbl,
                tc.tile_pool(name="xstage", bufs=3) as xstage,
                tc.tile_pool(name="xbstage", bufs=3) as xbstage,
                tc.tile_pool(name="xt", bufs=42) as xtpool,
                tc.tile_pool(name="rope", bufs=2) as rope,
                tc.tile_pool(name="vT", bufs=1) as vTpool,
                tc.tile_pool(name="qkvps", bufs=6, space="PSUM") as qkvps,
                tc.tile_pool(name="tps", bufs=2, space="PSUM") as tps,
            ):
                wsb = {}
                for nm, wt in (("q", wq_t), ("k", wk_t), ("v", wv_t)):
                    for d in range(NDT):
                        wtile = wpool.tile([128, EL], BF16, tag=f"w{nm}{d}", name=f"w{nm}{d}")
                        nc.gpsimd.dma_start(wtile[:], wt[d * 128:(d + 1) * 128, :])
                        wsb[(nm, d)] = wtile
                cos_sb = tbl.tile([128, S], F32, tag="cos", name="cos_sb")
                nc.gpsimd.dma_start(cos_sb[:], cos_t[:])
                sin_sb = tbl.tile([128, S], F32, tag="sin", name="sin_sb")
                nc.gpsimd.dma_start(sin_sb[:], sin_m[:])

                vT = [vTpool.tile([128, T], BF16, tag=f"vT{h}", name=f"vT{h}") for h in range(HL)]

                for tg in range(NTCH):
                    t0 = tg * TCH
                    # stage x chunk: load f32, cast to bf16, store to DRAM
                    for tt in range(TCH // 128):
                        r0 = t0 + tt * 128
                        xf = xstage.tile([128, D], F32, tag="xf", name="xf")
                        nc.gpsimd.dma_start(xf[:], x[r0:r0 + 128, :])
                        xb = xbstage.tile([128, D], BF16, tag="xb", name="xb")
                        nc.scalar.add(xb[:], xf[:], 0.0)
                        nc.gpsimd.dma_start(
                            xbf[tg][tt * 128:(tt + 1) * 128, :], xb[:])
                    # transpose chunk into SBUF: xt[d] = x[t0:t0+512, d*128:+128].T
                    xts = []
                    for d in range(NDT):
                        xtile = xtpool.tile([128, TCH], BF16, tag="xt", name="xt")
                        nc.sync.dma_start(
                            xtile[:], xbf[tg][:, d * 128:(d + 1) * 128],
                            transpose=True)
                        xts.append(xtile)
                        if _DEBUG and tg == 0 and d == 0:
                            nc.sync.dma_start(dbg["xt0"][:], xtile[:])
                    # QKV matmuls for this chunk
                    scol = (tg % NQC) * TCH  # s-offset within batch
                    for nm in ("q", "k", "v"):
                        for eh in range(HL):
                            ps = qkvps.tile([128, TCH], F32, tag="qkvps", name="qkvps")
                            for d in range(NDT):
                                nc.tensor.matmul(
                                    ps[:],
                                    wsb[(nm, d)][:, eh * 128:(eh + 1) * 128],
                                    xts[d][:],
                                    start=(d == 0), stop=(d == NDT - 1))
                            if nm == "v":
                                nc.vector.tensor_copy(
                                    vT[eh][:, t0:t0 + TCH], ps[:])
                            else:
                                dst = qT[eh] if nm == "q" else kT[eh]
                                tmp = rope.tile([128, TCH], F32, tag="ropetmp", name="ropetmp")
                                nc.vector.tensor_tensor(
                                    tmp[:], ps[:], cos_sb[:, scol:scol + TCH],
                                    OP.mult)
                                u = rope.tile([128, TCH], F32, tag="ropeu", name="ropeu")
                                nc.vector.tensor_tensor(
                                    u[0:64, :], ps[64:128, :],
                                    sin_sb[0:64, scol:scol + TCH], OP.mult)
                                nc.vector.tensor_tensor(
                                    u[64:128, :], ps[0:64, :],
                                    sin_sb[64:128, scol:scol + TCH], OP.mult)
                                nc.vector.tensor_tensor(
                                    dst[:, t0:t0 + TCH], tmp[:], u[:], OP.add)

                # transpose v tiles to natural [t, hd] layout (batched)
                for eh in range(HL):
                    for ktg in range(T // 128):
                        c0 = ktg * 128
                        tp = tps.tile([128, 128], BF16, tag="tps", name="tpsum")
                        nc.tensor.transpose(
                            tp[:], vT[eh][:, c0:c0 + 128], ident_sb[:])
                        nc.vector.tensor_copy(vnat[eh][:, c0:c0 + 128], tp[:])
                if _DEBUG:
                    nc.sync.dma_start(dbg["qT0"][:], qT[0][:])
                    nc.sync.dma_start(dbg["kT0"][:], kT[0][:])
                    nc.sync.dma_start(dbg["vn0"][:], vnat[0][:])

            # ---------- wo prefetch (overlaps SDPA) ----------
            with tc.tile_pool(name="wo", bufs=1) as wopool:
                wo_sb = []
                for d in range(NDT):
                    wtile = wopool.tile([128, D], BF16, tag=f"wo{d}", name=f"wo{d}")
                    nc.gpsimd.dma_start(wtile[:], wo_t[d * 128:(d + 1) * 128, :])
                    wo_sb.append(wtile)

                # ---------- phase C: SDPA per (batch, head) ----------
                with (
                    tc.tile_pool(name="E", bufs=NKT + 1) as epool,
                    tc.tile_pool(name="onorm", bufs=4) as onpool,
                    tc.tile_pool(name="rec", bufs=4) as recpool,
                    tc.tile_pool(name="sps", bufs=2, space="PSUM") as spool,
                    tc.tile_pool(name="ops", bufs=2, space="PSUM") as opool,
                    tc.tile_pool(name="dps", bufs=2, space="PSUM") as dpool,
                ):
                    for b in range(B):
                        q0 = b * S
                        for h in range(HL):
                            E = []
                            for kt in range(NKT):
                                e_t = epool.tile([128, S], BF16, tag="E", name="etile")
                                for half in range(2):
                                    sp = spool.tile([128, S // 2], F32, tag="sps", name="spsum")
                                    for qh in range(2):
                                        qc0 = q0 + half * 1024 + qh * 512
                                        nc.tensor.matmul(
                                            sp[:, qh * 512:(qh + 1) * 512],
                                            kT[h][:, q0 + kt * 128:q0 + (kt + 1) * 128],
                                            qT[h][:, qc0:qc0 + 512],
                                            start=True, stop=True)
                                    mcol = b * NKT + kt
                                    nc.scalar.activation(
                                        e_t[:, half * 1024:(half + 1) * 1024],
                                        sp[:], AF.Exp,
                                        bias=mask_sb[:, mcol:mcol + 1],
                                        scale=SCALE)
                                E.append(e_t)
                                if _DEBUG and b == 0 and h == 0 and kt == 0:
                                    nc.sync.dma_start(dbg["E0"][:], e_t[:])
                            for qc in range(NQC):
                                o_ps = opool.tile([128, 512], F32, tag="ops", name="opsum")
                                d_ps = dpool.tile([128, 512], F32, tag="dps", name="dpsum")
                                for kt in range(NKT):
                                    erhs = E[kt][:, qc * 512:(qc + 1) * 512]
                                    nc.tensor.matmul(
                                        o_ps[:],
                                        vnat[h][:, (b * NKT + kt) * 128:(b * NKT + kt + 1) * 128],
                                        erhs, start=(kt == 0), stop=(kt == NKT - 1))
                                for kt in range(NKT):
                                    erhs = E[kt][:, qc * 512:(qc + 1) * 512]
                                    nc.tensor.matmul(
                                        d_ps[:], ones_sb[:],
                                        erhs, start=(kt == 0), stop=(kt == NKT - 1))
                                rec = recpool.tile([128, 512], F32, tag="rec", name="rec")
                                nc.vector.reciprocal(rec[:], d_ps[:])
                                on = onpool.tile([128, 512], BF16, tag="on", name="onorm")
                                nc.vector.tensor_tensor(on[:], o_ps[:], rec[:], OP.mult)
                                j = b * NQC + qc
                                nc.scalar.dma_start(
                                    a2a_in[j, h * 128:(h + 1) * 128, :], on[:])
                                if _DEBUG and b == 0 and h == 0 and qc == 0:
                                    nc.sync.dma_start(dbg["on0"][:], on[:])
                                    nc.sync.dma_start(dbg["rec0"][:], rec[:])

                # ---------- AllToAll: head-split -> token-split ----------
                if _DEBUG:
                    nc.sync.dma_start(dbg["a2ain"][:], a2a_in[:])
                nc.gpsimd.collective_compute(
                    "AllToAll", OP.bypass,
                    replica_groups=[list(range(NCORES))],
                    ins=[a2a_in.opt()],
                    outs=[a2a_out.opt()],
                )

                # ---------- phase D: output projection for my 512 tokens ----------
                if _DEBUG:
                    nc.sync.dma_start(dbg["a2aout"][:], a2a_out[:])
                with (
                    tc.tile_pool(name="ot", bufs=1) as otpool,
                    tc.tile_pool(name="ysb", bufs=4) as ypool,
                    tc.tile_pool(name="yps", bufs=8, space="PSUM") as ypsp,
                ):
                    ot_sb = []
                    for d in range(NDT):
                        otile = otpool.tile([128, TL], BF16, tag=f"ot{d}", name=f"ot{d}")
                        blk, row = d // 2, (d % 2) * 128
                        nc.sync.dma_start(
                            otile[:], a2a_out[blk, row:row + 128, :])
                        ot_sb.append(otile)
                        if _DEBUG and d == 0:
                            nc.sync.dma_start(dbg["ot0"][:], otile[:])
                    for tt in range(TL // 128):
                        yps = [ypsp.tile([128, 512], F32, tag="yps", name="ypsum")
                               for _ in range(4)]
                        for d in range(NDT):
                            for eo in range(4):
                                nc.tensor.matmul(
                                    yps[eo][:],
                                    ot_sb[d][:, tt * 128:(tt + 1) * 128],
                                    wo_sb[d][:, eo * 512:(eo + 1) * 512],
                                    start=(d == 0), stop=(d == NDT - 1))
                        for eo in range(4):
                            ysb = ypool.tile([128, 512], F32, tag="ysb", name="ysb")
                            nc.vector.tensor_copy(ysb[:], yps[eo][:])
                            if _DEBUG and tt == 0 and eo == 0:
                                nc.sync.dma_start(dbg["y0"][:], ysb[:])
                            nc.gpsimd.dma_start(
                                out[tt * 128:(tt + 1) * 128,
                                    eo * 512:(eo + 1) * 512], ysb[:])

    nc.compile()
    return nc


def _prep_in_maps(x, cos, sin, attn_mask, wq, wk, wv, wo):
    xf = np.ascontiguousarray(x.reshape(T, D).astype(np.float32, copy=False))
    cosT = np.ascontiguousarray(np.asarray(cos[0], np.float32).T)   # [HD, S]
    sinT = np.asarray(sin[0], np.float32).T
    sin_m = np.ascontiguousarray(
        np.concatenate([-sinT[:64], sinT[64:]], axis=0))            # [HD, S]
    mask_t = np.ascontiguousarray(
        np.asarray(attn_mask, np.float32).reshape(B * NKT, 128).T)  # [128, 32]
    wo_t = np.ascontiguousarray(np.asarray(wo, np.float32).T.astype(bf16))
    in_maps = []
    for i in range(NCORES):
        sl = slice(i * EL, (i + 1) * EL)
        in_maps.append({
            "x": xf,
            "wq_t": np.ascontiguousarray(np.asarray(wq, np.float32)[sl].T.astype(bf16)),
            "wk_t": np.ascontiguousarray(np.asarray(wk, np.float32)[sl].T.astype(bf16)),
            "wv_t": np.ascontiguousarray(np.asarray(wv, np.float32)[sl].T.astype(bf16)),
            "wo_t": wo_t,
            "cos_t": cosT,
            "sin_m": sin_m,
            "mask_t": mask_t,
        })
    return in_maps


def kernel(x, cos, sin, attn_mask, wq, wk, wv, wo, _trace=False):
    if "nc" not in _CACHE:
        _CACHE["nc"] = _build()
    nc = _CACHE["nc"]
    in_maps = _prep_in_maps(x, cos, sin, attn_mask, wq, wk, wv, wo)
    res = run_bass_kernel_spmd(nc, in_maps, core_ids=list(range(NCORES)),
                               trace=_trace)
    _CACHE["last_result"] = res
    y = np.concatenate([np.asarray(res.results[i]["out"], np.float32)
                        for i in range(NCORES)], axis=0)
    return y.reshape(B, S, D)


# revision 11
# speedup vs baseline: 1.0691x; 1.0691x over previous
"""Trainium2 Bass kernel for multi-head attention with RoPE (B=2, S=2048,
D=2048, H=16), distributed over 8 NeuronCores with head tensor-parallelism
and an AllToAll to switch to token-parallelism for the output projection.

kernel(**inputs) takes the full unsharded inputs (as produced by the
reference setup_inputs) and returns the full [2, 2048, 2048] f32 output.
"""
import numpy as np
import ml_dtypes
from concourse import bass, bacc, tile, mybir
from concourse.bass_utils import run_bass_kernel_spmd

bf16 = ml_dtypes.bfloat16
BF16 = mybir.dt.bfloat16
F32 = mybir.dt.float32
AF = mybir.ActivationFunctionType
OP = mybir.AluOpType

B, S, D, H = 2, 2048, 2048, 16
HD = 128                 # head dim
NCORES = 8
HL = H // NCORES         # heads per core = 2
EL = HL * HD             # local projection width = 256
T = B * S                # 4096 flattened tokens
TCH = 512                # token chunk for QKV phase
NTCH = T // TCH          # 8
NKT = S // 128           # 16 key tiles per batch
NQC = S // 512           # 4 query chunks per batch
NDT = D // 128           # 16 contraction tiles
TL = T // NCORES         # 512 tokens per core after AllToAll
SCALE = float(1.0 / np.sqrt(128.0))

_CACHE = {}
_DEBUG = False


def _build():
    nc = bacc.Bacc("TRN2", target_bir_lowering=False, num_devices=NCORES)
    dbg = {}
    if _DEBUG:
        dbg["qT0"] = nc.dram_tensor("dbg_qT0", [128, T], BF16, kind="ExternalOutput")
        dbg["kT0"] = nc.dram_tensor("dbg_kT0", [128, T], BF16, kind="ExternalOutput")
        dbg["vn0"] = nc.dram_tensor("dbg_vn0", [128, T], BF16, kind="ExternalOutput")
        dbg["xt0"] = nc.dram_tensor("dbg_xt0", [128, TCH], BF16, kind="ExternalOutput")
        dbg["E0"] = nc.dram_tensor("dbg_E0", [128, S], BF16, kind="ExternalOutput")
        dbg["on0"] = nc.dram_tensor("dbg_on0", [128, 512], BF16, kind="ExternalOutput")
        dbg["rec0"] = nc.dram_tensor("dbg_rec0", [128, 512], F32, kind="ExternalOutput")
        dbg["ot0"] = nc.dram_tensor("dbg_ot0", [128, TL], BF16, kind="ExternalOutput")
        dbg["a2ain"] = nc.dram_tensor("dbg_a2ain", [NCORES, EL, TL], BF16, kind="ExternalOutput")
        dbg["a2aout"] = nc.dram_tensor("dbg_a2aout", [NCORES, EL, TL], BF16, kind="ExternalOutput")
        dbg["y0"] = nc.dram_tensor("dbg_y0", [128, 512], F32, kind="ExternalOutput")

    x = nc.dram_tensor("x", [T, D], F32, kind="ExternalInput")
    wq_t = nc.dram_tensor("wq_t", [D, EL], BF16, kind="ExternalInput")
    wk_t = nc.dram_tensor("wk_t", [D, EL], BF16, kind="ExternalInput")
    wv_t = nc.dram_tensor("wv_t", [D, EL], BF16, kind="ExternalInput")
    wo_t = nc.dram_tensor("wo_t", [D, D], BF16, kind="ExternalInput")
    cos_t = nc.dram_tensor("cos_t", [HD, S], F32, kind="ExternalInput")
    sin_m = nc.dram_tensor("sin_m", [HD, S], F32, kind="ExternalInput")
    mask_t = nc.dram_tensor("mask_t", [128, B * NKT], F32, kind="ExternalInput")
    out = nc.dram_tensor("out", [TL, D], F32, kind="ExternalOutput")

    ident_dram = nc.inline_tensor(np.eye(128, dtype=bf16), name="ident")
    ones_dram = nc.inline_tensor(np.ones((128, 128), dtype=bf16), name="ones")

    with tile.TileContext(nc) as tc:
        with (
            # ---------- persistent pools (whole kernel) ----------
            tc.tile_pool(name="dram", bufs=1, space="DRAM") as dram,
            tc.tile_pool(name="consts", bufs=1) as consts,
            tc.tile_pool(name="qkv_keep", bufs=1) as keep,
        ):
            xbf = [dram.tile([1024, D], BF16, tag=f"xbf{c}", name=f"xbf{c}")
                   for c in range(T // 1024)]
            a2a_in = dram.tile([NCORES, EL, TL], BF16, tag="a2a_in", name="a2a_in")
            a2a_out = dram.tile([NCORES, EL, TL], BF16, tag="a2a_out", name="a2a_out")

            ident_sb = consts.tile([128, 128], BF16, tag="ident", name="ident_sb")
            nc.sync.dma_start(ident_sb[:], ident_dram[:])
            ones_sb = consts.tile([128, 128], BF16, tag="ones", name="ones_sb")
            nc.sync.dma_start(ones_sb[:], ones_dram[:])
            mask_sb = consts.tile([128, B * NKT], F32, tag="mask", name="mask_sb")
            nc.sync.dma_start(mask_sb[:], mask_t[:])

            # persistent per-head tensors ([hd, t] layouts; v natural packed
            # as 32 [t=128, hd=128] tiles along free dim)
            qT = [keep.tile([128, T], BF16, tag=f"qT{h}", name=f"qT{h}") for h in range(HL)]
            kT = [keep.tile([128, T], BF16, tag=f"kT{h}", name=f"kT{h}") for h in range(HL)]
            vnat = [keep.tile([128, T], BF16, tag=f"vn{h}", name=f"vn{h}") for h in range(HL)]

            # ---------- phase A+B: x prep + QKV projections + RoPE ----------
            with (
                tc.tile_pool(name="wsb", bufs=1) as wpool,
                tc.tile_pool(name="tables", bufs=1) as tbl,
                tc.tile_pool(name="xstage", bufs=4) as xstage,
                tc.tile_pool(name="xbstage", bufs=3) as xbstage,
                tc.tile_pool(name="xt", bufs=20) as xtpool,
                tc.tile_pool(name="rope", bufs=2) as rope,
                tc.tile_pool(name="vT", bufs=1) as vTpool,
                tc.tile_pool(name="qkvps", bufs=6, space="PSUM") as qkvps,
                tc.tile_pool(name="tps", bufs=2, space="PSUM") as tps,
            ):
                wsb = {}
                for nm, wt in (("q", wq_t), ("k", wk_t), ("v", wv_t)):
                    for d in range(NDT):
                        wtile = wpool.tile([128, EL], BF16, tag=f"w{nm}{d}", name=f"w{nm}{d}")
                        nc.gpsimd.dma_start(wtile[:], wt[d * 128:(d + 1) * 128, :])
                        wsb[(nm, d)] = wtile
                cos_sb = tbl.tile([128, S], F32, tag="cos", name="cos_sb")
                nc.gpsimd.dma_start(cos_sb[:], cos_t[:])
                sin_sb = tbl.tile([128, S], F32, tag="sin", name="sin_sb")
                nc.gpsimd.dma_start(sin_sb[:], sin_m[:])

                vT = [vTpool.tile([128, T], BF16, tag=f"vT{h}", name=f"vT{h}") for h in range(HL)]

                TG2 = 1024
                for tg2 in range(T // TG2):   # 1024-token groups
                    g0 = tg2 * TG2
                    # stage x: load f32, cast to bf16, store to DRAM scratch
                    for tt in range(TG2 // 128):
                        r0 = g0 + tt * 128
                        xf = xstage.tile([128, D], F32, tag="xf", name="xf")
                        nc.gpsimd.dma_start(xf[:], x[r0:r0 + 128, :])
                        xb = xbstage.tile([128, D], BF16, tag="xb", name="xb")
                        if tt % 2 == 0:
                            nc.scalar.add(xb[:], xf[:], 0.0)
                        else:
                            nc.vector.tensor_copy(xb[:], xf[:])
                        nc.gpsimd.dma_start(
                            xbf[tg2][tt * 128:(tt + 1) * 128, :], xb[:])
                    # transpose group into SBUF: xt[d] = xbf[:, d*128:+128].T
                    xts = []
                    for d in range(NDT):
                        xtile = xtpool.tile([128, TG2], BF16, tag="xt", name="xt")
                        nc.sync.dma_start(
                            xtile[:], xbf[tg2][:, d * 128:(d + 1) * 128],
                            transpose=True)
                        xts.append(xtile)
                    # QKV matmuls per 512-token half
                    for half in range(2):
                        t0 = g0 + half * TCH
                        hs = half * TCH
                        scol = ((t0 // TCH) % NQC) * TCH
                        for nm in ("q", "k", "v"):
                            for eh in range(HL):
                                ps = qkvps.tile([128, TCH], F32, tag="qkvps", name="qkvps")
                                for d in range(NDT):
                                    nc.tensor.matmul(
                                        ps[:],
                                        wsb[(nm, d)][:, eh * 128:(eh + 1) * 128],
                                        xts[d][:, hs:hs + TCH],
                                        start=(d == 0), stop=(d == NDT - 1))
                                if nm == "v":
                                    nc.vector.tensor_copy(
                                        vT[eh][:, t0:t0 + TCH], ps[:])
                                else:
                                    dst = qT[eh] if nm == "q" else kT[eh]
                                    tmp = rope.tile([128, TCH], F32, tag="ropetmp", name="ropetmp")
                                    nc.vector.tensor_tensor(
                                        tmp[:], ps[:], cos_sb[:, scol:scol + TCH],
                                        OP.mult)
                                    u = rope.tile([128, TCH], F32, tag="ropeu", name="ropeu")
                                    nc.vector.tensor_tensor(
                                        u[0:64, :], ps[64:128, :],
                                        sin_sb[0:64, scol:scol + TCH], OP.mult)
                                    nc.vector.tensor_tensor(
                                        u[64:128, :], ps[0:64, :],
                                        sin_sb[64:128, scol:scol + TCH], OP.mult)
                                    nc.vector.tensor_tensor(
                                        dst[:, t0:t0 + TCH], tmp[:], u[:], OP.add)
                # transpose v tiles to natural [t, hd] layout (batched)
                for eh in range(HL):
                    for ktg in range(T // 128):
                        c0 = ktg * 128
                        tp = tps.tile([128, 128], BF16, tag="tps", name="tpsum")
                        nc.tensor.transpose(
                            tp[:], vT[eh][:, c0:c0 + 128], ident_sb[:])
                        nc.vector.tensor_copy(vnat[eh][:, c0:c0 + 128], tp[:])
                if _DEBUG:
                    nc.sync.dma_start(dbg["qT0"][:], qT[0][:])
                    nc.sync.dma_start(dbg["kT0"][:], kT[0][:])
                    nc.sync.dma_start(dbg["vn0"][:], vnat[0][:])

            # ---------- wo prefetch (overlaps SDPA) ----------
            with tc.tile_pool(name="wo", bufs=1) as wopool:
                wo_sb = []
                for d in range(NDT):
                    wtile = wopool.tile([128, D], BF16, tag=f"wo{d}", name=f"wo{d}")
                    nc.gpsimd.dma_start(wtile[:], wo_t[d * 128:(d + 1) * 128, :])
                    wo_sb.append(wtile)

                # ---------- phase C: SDPA per (batch, head) ----------
                with (
                    tc.tile_pool(name="E", bufs=NKT + 1) as epool,
                    tc.tile_pool(name="onorm", bufs=4) as onpool,
                    tc.tile_pool(name="rec", bufs=4) as recpool,
                    tc.tile_pool(name="sps", bufs=2, space="PSUM") as spool,
                    tc.tile_pool(name="ops", bufs=2, space="PSUM") as opool,
                    tc.tile_pool(name="dps", bufs=2, space="PSUM") as dpool,
                ):
                    for b in range(B):
                        q0 = b * S
                        for h in range(HL):
                            E = []
                            for kt in range(NKT):
                                e_t = epool.tile([128, S], BF16, tag="E", name="etile")
                                for half in range(2):
                                    sp = spool.tile([128, S // 2], F32, tag="sps", name="spsum")
                                    for qh in range(2):
                                        qc0 = q0 + half * 1024 + qh * 512
                                        nc.tensor.matmul(
                                            sp[:, qh * 512:(qh + 1) * 512],
                                            kT[h][:, q0 + kt * 128:q0 + (kt + 1) * 128],
                                            qT[h][:, qc0:qc0 + 512],
                                            start=True, stop=True)
                                    mcol = b * NKT + kt
                                    nc.scalar.activation(
                                        e_t[:, half * 1024:(half + 1) * 1024],
                                        sp[:], AF.Exp,
                                        bias=mask_sb[:, mcol:mcol + 1],
                                        scale=SCALE)
                                E.append(e_t)
                                if _DEBUG and b == 0 and h == 0 and kt == 0:
                                    nc.sync.dma_start(dbg["E0"][:], e_t[:])
                            for qc in range(NQC):
                                o_ps = opool.tile([128, 512], F32, tag="ops", name="opsum")
                                d_ps = dpool.tile([128, 512], F32, tag="dps", name="dpsum")
                                for kt in range(NKT):
                                    erhs = E[kt][:, qc * 512:(qc + 1) * 512]
                                    nc.tensor.matmul(
                                        o_ps[:],
                                        vnat[h][:, (b * NKT + kt) * 128:(b * NKT + kt + 1) * 128],
                                        erhs, start=(kt == 0), stop=(kt == NKT - 1))
                                for kt in range(NKT):
                                    erhs = E[kt][:, qc * 512:(qc + 1) * 512]
                                    nc.tensor.matmul(
                                        d_ps[:], ones_sb[:],
                                        erhs, start=(kt == 0), stop=(kt == NKT - 1))
                                rec = recpool.tile([128, 512], F32, tag="rec", name="rec")
                                nc.vector.reciprocal(rec[:], d_ps[:])
                                on = onpool.tile([128, 512], BF16, tag="on", name="onorm")
                                nc.vector.tensor_tensor(on[:], o_ps[:], rec[:], OP.mult)
                                j = b * NQC + qc
                                nc.scalar.dma_start(
                                    a2a_in[j, h * 128:(h + 1) * 128, :], on[:])
                                if _DEBUG and b == 0 and h == 0 and qc == 0:
                                    nc.sync.dma_start(dbg["on0"][:], on[:])
                                    nc.sync.dma_start(dbg["rec0"][:], rec[:])

                # ---------- AllToAll: head-split -> token-split ----------
                if _DEBUG:
                    nc.sync.dma_start(dbg["a2ain"][:], a2a_in[:])
                nc.gpsimd.collective_compute(
                    "AllToAll", OP.bypass,
                    replica_groups=[list(range(NCORES))],
                    ins=[a2a_in.opt()],
                    outs=[a2a_out.opt()],
                )

                # ---------- phase D: output projection for my 512 tokens ----------
                if _DEBUG:
                    nc.sync.dma_start(dbg["a2aout"][:], a2a_out[:])
                with (
                    tc.tile_pool(name="ot", bufs=1) as otpool,
                    tc.tile_pool(name="ysb", bufs=4) as ypool,
                    tc.tile_pool(name="yps", bufs=8, space="PSUM") as ypsp,
                ):
                    ot_sb = []
                    for d in range(NDT):
                        otile = otpool.tile([128, TL], BF16, tag=f"ot{d}", name=f"ot{d}")
                        blk, row = d // 2, (d % 2) * 128
                        nc.sync.dma_start(
                            otile[:], a2a_out[blk, row:row + 128, :])
                        ot_sb.append(otile)
                        if _DEBUG and d == 0:
                            nc.sync.dma_start(dbg["ot0"][:], otile[:])
                    for tt in range(TL // 128):
                        yps = [ypsp.tile([128, 512], F32, tag="yps", name="ypsum")
                               for _ in range(4)]
                        for d in range(NDT):
                            for eo in range(4):
                                nc.tensor.matmul(
                                    yps[eo][:],
                                    ot_sb[d][:, tt * 128:(tt + 1) * 128],
                                    wo_sb[d][:, eo * 512:(eo + 1) * 512],
                                    start=(d == 0), stop=(d == NDT - 1))
                        for eo in range(4):
                            ysb = ypool.tile([128, 512], F32, tag="ysb", name="ysb")
                            nc.vector.tensor_copy(ysb[:], yps[eo][:])
                            if _DEBUG and tt == 0 and eo == 0:
                                nc.sync.dma_start(dbg["y0"][:], ysb[:])
                            nc.gpsimd.dma_start(
                                out[tt * 128:(tt + 1) * 128,
                                    eo * 512:(eo + 1) * 512], ysb[:])

    nc.compile()
    return nc


def _prep_in_maps(x, cos, sin, attn_mask, wq, wk, wv, wo):
    xf = np.ascontiguousarray(x.reshape(T, D).astype(np.float32, copy=False))
    cosT = np.ascontiguousarray(np.asarray(cos[0], np.float32).T)   # [HD, S]
    sinT = np.asarray(sin[0], np.float32).T
    sin_m = np.ascontiguousarray(
        np.concatenate([-sinT[:64], sinT[64:]], axis=0))            # [HD, S]
    mask_t = np.ascontiguousarray(
        np.asarray(attn_mask, np.float32).reshape(B * NKT, 128).T)  # [128, 32]
    wo_t = np.ascontiguousarray(np.asarray(wo, np.float32).T.astype(bf16))
    in_maps = []
    for i in range(NCORES):
        sl = slice(i * EL, (i + 1) * EL)
        in_maps.append({
            "x": xf,
            "wq_t": np.ascontiguousarray(np.asarray(wq, np.float32)[sl].T.astype(bf16)),
            "wk_t": np.ascontiguousarray(np.asarray(wk, np.float32)[sl].T.astype(bf16)),
            "wv_t": np.ascontiguousarray(np.asarray(wv, np.float32)[sl].T.astype(bf16)),
            "wo_t": wo_t,
            "cos_t": cosT,
            "sin_m": sin_m,
            "mask_t": mask_t,
        })
    return in_maps


def kernel(x, cos, sin, attn_mask, wq, wk, wv, wo, _trace=False):
    if "nc" not in _CACHE:
        _CACHE["nc"] = _build()
    nc = _CACHE["nc"]
    in_maps = _prep_in_maps(x, cos, sin, attn_mask, wq, wk, wv, wo)
    res = run_bass_kernel_spmd(nc, in_maps, core_ids=list(range(NCORES)),
                               trace=_trace)
    _CACHE["last_result"] = res
    y = np.concatenate([np.asarray(res.results[i]["out"], np.float32)
                        for i in range(NCORES)], axis=0)
    return y.reshape(B, S, D)


# revision 13
# speedup vs baseline: 1.1128x; 1.0409x over previous
"""Trainium2 Bass kernel for multi-head attention with RoPE (B=2, S=2048,
D=2048, H=16), distributed over 8 NeuronCores with head tensor-parallelism
and an AllToAll to switch to token-parallelism for the output projection.

kernel(**inputs) takes the full unsharded inputs (as produced by the
reference setup_inputs) and returns the full [2, 2048, 2048] f32 output.
"""
import numpy as np
import ml_dtypes
from concourse import bass, bacc, tile, mybir
from concourse.bass_utils import run_bass_kernel_spmd

bf16 = ml_dtypes.bfloat16
BF16 = mybir.dt.bfloat16
F32 = mybir.dt.float32
AF = mybir.ActivationFunctionType
OP = mybir.AluOpType

B, S, D, H = 2, 2048, 2048, 16
HD = 128                 # head dim
NCORES = 8
HL = H // NCORES         # heads per core = 2
EL = HL * HD             # local projection width = 256
T = B * S                # 4096 flattened tokens
TCH = 512                # token chunk for QKV phase
NTCH = T // TCH          # 8
NKT = S // 128           # 16 key tiles per batch
NQC = S // 512           # 4 query chunks per batch
NDT = D // 128           # 16 contraction tiles
TL = T // NCORES         # 512 tokens per core after AllToAll
SCALE = float(1.0 / np.sqrt(128.0))

_CACHE = {}
_DEBUG = False


def _build():
    nc = bacc.Bacc("TRN2", target_bir_lowering=False, num_devices=NCORES)
    dbg = {}
    if _DEBUG:
        dbg["qT0"] = nc.dram_tensor("dbg_qT0", [128, T], BF16, kind="ExternalOutput")
        dbg["kT0"] = nc.dram_tensor("dbg_kT0", [128, T], BF16, kind="ExternalOutput")
        dbg["vn0"] = nc.dram_tensor("dbg_vn0", [128, T], BF16, kind="ExternalOutput")
        dbg["xt0"] = nc.dram_tensor("dbg_xt0", [128, TCH], BF16, kind="ExternalOutput")
        dbg["E0"] = nc.dram_tensor("dbg_E0", [128, S], BF16, kind="ExternalOutput")
        dbg["on0"] = nc.dram_tensor("dbg_on0", [128, 512], BF16, kind="ExternalOutput")
        dbg["rec0"] = nc.dram_tensor("dbg_rec0", [128, 512], F32, kind="ExternalOutput")
        dbg["ot0"] = nc.dram_tensor("dbg_ot0", [128, TL], BF16, kind="ExternalOutput")
        dbg["a2ain"] = nc.dram_tensor("dbg_a2ain", [NCORES, EL, TL], BF16, kind="ExternalOutput")
        dbg["a2aout"] = nc.dram_tensor("dbg_a2aout", [NCORES, EL, TL], BF16, kind="ExternalOutput")
        dbg["y0"] = nc.dram_tensor("dbg_y0", [128, 512], F32, kind="ExternalOutput")

    x = nc.dram_tensor("x", [T, D], F32, kind="ExternalInput")
    wq_t = nc.dram_tensor("wq_t", [D, EL], BF16, kind="ExternalInput")
    wk_t = nc.dram_tensor("wk_t", [D, EL], BF16, kind="ExternalInput")
    wv_t = nc.dram_tensor("wv_t", [D, EL], BF16, kind="ExternalInput")
    wo_t = nc.dram_tensor("wo_t", [D, D], BF16, kind="ExternalInput")
    cos_t = nc.dram_tensor("cos_t", [HD, S], F32, kind="ExternalInput")
    sin_m = nc.dram_tensor("sin_m", [HD, S], F32, kind="ExternalInput")
    mask_t = nc.dram_tensor("mask_t", [128, B * NKT], F32, kind="ExternalInput")
    out = nc.dram_tensor("out", [TL, D], F32, kind="ExternalOutput")

    ident_dram = nc.inline_tensor(np.eye(128, dtype=bf16), name="ident")
    ones_dram = nc.inline_tensor(np.ones((128, 128), dtype=bf16), name="ones")

    with tile.TileContext(nc) as tc:
        with (
            # ---------- persistent pools (whole kernel) ----------
            tc.tile_pool(name="dram", bufs=1, space="DRAM") as dram,
            tc.tile_pool(name="consts", bufs=1) as consts,
            tc.tile_pool(name="qkv_keep", bufs=1) as keep,
        ):
            xbf = [dram.tile([1024, D], BF16, tag=f"xbf{c}", name=f"xbf{c}")
                   for c in range(T // 1024)]
            a2a_in = [dram.tile([NCORES, HD, TL], BF16, tag=f"a2a_in{h}", name=f"a2a_in{h}")
                      for h in range(HL)]
            a2a_out = [dram.tile([NCORES, HD, TL], BF16, tag=f"a2a_out{h}", name=f"a2a_out{h}")
                       for h in range(HL)]

            ident_sb = consts.tile([128, 128], BF16, tag="ident", name="ident_sb")
            nc.sync.dma_start(ident_sb[:], ident_dram[:])
            ones_sb = consts.tile([128, 128], BF16, tag="ones", name="ones_sb")
            nc.sync.dma_start(ones_sb[:], ones_dram[:])
            mask_sb = consts.tile([128, B * NKT], F32, tag="mask", name="mask_sb")
            nc.sync.dma_start(mask_sb[:], mask_t[:])

            # persistent per-head tensors ([hd, t] layouts; v natural packed
            # as 32 [t=128, hd=128] tiles along free dim)
            qT = [keep.tile([128, T], BF16, tag=f"qT{h}", name=f"qT{h}") for h in range(HL)]
            kT = [keep.tile([128, T], BF16, tag=f"kT{h}", name=f"kT{h}") for h in range(HL)]
            vnat = [keep.tile([128, T], BF16, tag=f"vn{h}", name=f"vn{h}") for h in range(HL)]

            # ---------- phase A+B: x prep + QKV projections + RoPE ----------
            with (
                tc.tile_pool(name="wsb", bufs=1) as wpool,
                tc.tile_pool(name="tables", bufs=1) as tbl,
                tc.tile_pool(name="xstage", bufs=4) as xstage,
                tc.tile_pool(name="xbstage", bufs=3) as xbstage,
                tc.tile_pool(name="xt", bufs=20) as xtpool,
                tc.tile_pool(name="rope", bufs=2) as rope,
                tc.tile_pool(name="vT", bufs=1) as vTpool,
                tc.tile_pool(name="qkvps", bufs=6, space="PSUM") as qkvps,
                tc.tile_pool(name="tps", bufs=2, space="PSUM") as tps,
            ):
                wsb = {}
                for nm, wt in (("q", wq_t), ("k", wk_t), ("v", wv_t)):
                    for d in range(NDT):
                        wtile = wpool.tile([128, EL], BF16, tag=f"w{nm}{d}", name=f"w{nm}{d}")
                        nc.gpsimd.dma_start(wtile[:], wt[d * 128:(d + 1) * 128, :])
                        wsb[(nm, d)] = wtile
                cos_sb = tbl.tile([128, S], F32, tag="cos", name="cos_sb")
                nc.gpsimd.dma_start(cos_sb[:], cos_t[:])
                sin_sb = tbl.tile([128, S], F32, tag="sin", name="sin_sb")
                nc.gpsimd.dma_start(sin_sb[:], sin_m[:])

                vT = [vTpool.tile([128, T], BF16, tag=f"vT{h}", name=f"vT{h}") for h in range(HL)]

                TG2 = 1024
                for tg2 in range(T // TG2):   # 1024-token groups
                    g0 = tg2 * TG2
                    # stage x: load f32, cast to bf16, store to DRAM scratch
                    for tt in range(TG2 // 128):
                        r0 = g0 + tt * 128
                        xf = xstage.tile([128, D], F32, tag="xf", name="xf")
                        nc.gpsimd.dma_start(xf[:], x[r0:r0 + 128, :])
                        xb = xbstage.tile([128, D], BF16, tag="xb", name="xb")
                        if tt % 2 == 0:
                            nc.scalar.add(xb[:], xf[:], 0.0)
                        else:
                            nc.vector.tensor_copy(xb[:], xf[:])
                        nc.gpsimd.dma_start(
                            xbf[tg2][tt * 128:(tt + 1) * 128, :], xb[:])
                    # transpose group into SBUF: xt[d] = xbf[:, d*128:+128].T
                    xts = []
                    for d in range(NDT):
                        xtile = xtpool.tile([128, TG2], BF16, tag="xt", name="xt")
                        nc.sync.dma_start(
                            xtile[:], xbf[tg2][:, d * 128:(d + 1) * 128],
                            transpose=True)
                        xts.append(xtile)
                    # QKV matmuls per 512-token half
                    for half in range(2):
                        t0 = g0 + half * TCH
                        hs = half * TCH
                        scol = ((t0 // TCH) % NQC) * TCH
                        for nm in ("q", "k", "v"):
                            for eh in range(HL):
                                ps = qkvps.tile([128, TCH], F32, tag="qkvps", name="qkvps")
                                for d in range(NDT):
                                    nc.tensor.matmul(
                                        ps[:],
                                        wsb[(nm, d)][:, eh * 128:(eh + 1) * 128],
                                        xts[d][:, hs:hs + TCH],
                                        start=(d == 0), stop=(d == NDT - 1))
                                if nm == "v":
                                    nc.vector.tensor_copy(
                                        vT[eh][:, t0:t0 + TCH], ps[:])
                                else:
                                    dst = qT[eh] if nm == "q" else kT[eh]
                                    tmp = rope.tile([128, TCH], F32, tag="ropetmp", name="ropetmp")
                                    nc.vector.tensor_tensor(
                                        tmp[:], ps[:], cos_sb[:, scol:scol + TCH],
                                        OP.mult)
                                    u = rope.tile([128, TCH], F32, tag="ropeu", name="ropeu")
                                    nc.vector.tensor_tensor(
                                        u[0:64, :], ps[64:128, :],
                                        sin_sb[0:64, scol:scol + TCH], OP.mult)
                                    nc.vector.tensor_tensor(
                                        u[64:128, :], ps[0:64, :],
                                        sin_sb[64:128, scol:scol + TCH], OP.mult)
                                    nc.vector.tensor_tensor(
                                        dst[:, t0:t0 + TCH], tmp[:], u[:], OP.add)
                # transpose v tiles to natural [t, hd] layout (batched)
                for eh in range(HL):
                    for ktg in range(T // 128):
                        c0 = ktg * 128
                        tp = tps.tile([128, 128], BF16, tag="tps", name="tpsum")
                        nc.tensor.transpose(
                            tp[:], vT[eh][:, c0:c0 + 128], ident_sb[:])
                        nc.vector.tensor_copy(vnat[eh][:, c0:c0 + 128], tp[:])
                if _DEBUG:
                    nc.sync.dma_start(dbg["qT0"][:], qT[0][:])
                    nc.sync.dma_start(dbg["kT0"][:], kT[0][:])
                    nc.sync.dma_start(dbg["vn0"][:], vnat[0][:])

            # ---------- wo prefetch (overlaps SDPA) ----------
            with tc.tile_pool(name="wo", bufs=1) as wopool:
                wo_sb = []
                for d in range(NDT):
                    wtile = wopool.tile([128, D], BF16, tag=f"wo{d}", name=f"wo{d}")
                    nc.gpsimd.dma_start(wtile[:], wo_t[d * 128:(d + 1) * 128, :])
                    wo_sb.append(wtile)

                # ---------- phase C: SDPA per (batch, head) ----------
                with (
                    tc.tile_pool(name="E", bufs=NKT + 1) as epool,
                    tc.tile_pool(name="onorm", bufs=4) as onpool,
                    tc.tile_pool(name="rec", bufs=4) as recpool,
                    tc.tile_pool(name="sps", bufs=2, space="PSUM") as spool,
                    tc.tile_pool(name="ops", bufs=2, space="PSUM") as opool,
                    tc.tile_pool(name="dps", bufs=2, space="PSUM") as dpool,
                ):
                    for h in range(HL):
                        for b in range(B):
                            q0 = b * S
                            E = []
                            for kt in range(NKT):
                                e_t = epool.tile([128, S], BF16, tag="E", name="etile")
                                for half in range(2):
                                    sp = spool.tile([128, S // 2], F32, tag="sps", name="spsum")
                                    for qh in range(2):
                                        qc0 = q0 + half * 1024 + qh * 512
                                        nc.tensor.matmul(
                                            sp[:, qh * 512:(qh + 1) * 512],
                                            kT[h][:, q0 + kt * 128:q0 + (kt + 1) * 128],
                                            qT[h][:, qc0:qc0 + 512],
                                            start=True, stop=True)
                                    mcol = b * NKT + kt
                                    nc.scalar.activation(
                                        e_t[:, half * 1024:(half + 1) * 1024],
                                        sp[:], AF.Exp,
                                        bias=mask_sb[:, mcol:mcol + 1],
                                        scale=SCALE)
                                E.append(e_t)
                                if _DEBUG and b == 0 and h == 0 and kt == 0:
                                    nc.sync.dma_start(dbg["E0"][:], e_t[:])
                            for qc in range(NQC):
                                o_ps = opool.tile([128, 512], F32, tag="ops", name="opsum")
                                d_ps = dpool.tile([128, 512], F32, tag="dps", name="dpsum")
                                for kt in range(NKT):
                                    erhs = E[kt][:, qc * 512:(qc + 1) * 512]
                                    nc.tensor.matmul(
                                        o_ps[:],
                                        vnat[h][:, (b * NKT + kt) * 128:(b * NKT + kt + 1) * 128],
                                        erhs, start=(kt == 0), stop=(kt == NKT - 1))
                                for kt in range(NKT):
                                    erhs = E[kt][:, qc * 512:(qc + 1) * 512]
                                    nc.tensor.matmul(
                                        d_ps[:], ones_sb[:],
                                        erhs, start=(kt == 0), stop=(kt == NKT - 1))
                                rec = recpool.tile([128, 512], F32, tag="rec", name="rec")
                                nc.vector.reciprocal(rec[:], d_ps[:])
                                on = onpool.tile([128, 512], BF16, tag="on", name="onorm")
                                nc.vector.tensor_tensor(on[:], o_ps[:], rec[:], OP.mult)
                                j = b * NQC + qc
                                nc.scalar.dma_start(a2a_in[h][j, :, :], on[:])
                                if _DEBUG and b == 0 and h == 0 and qc == 0:
                                    nc.sync.dma_start(dbg["on0"][:], on[:])
                                    nc.sync.dma_start(dbg["rec0"][:], rec[:])
                        # fire this head's AllToAll (head 0's overlaps head 1 SDPA)
                        nc.gpsimd.collective_compute(
                            "AllToAll", OP.bypass,
                            replica_groups=[list(range(NCORES))],
                            ins=[a2a_in[h].opt()],
                            outs=[a2a_out[h].opt()],
                        )

                # ---------- phase D: output projection for my 512 tokens ----------
                with (
                    tc.tile_pool(name="ot", bufs=1) as otpool,
                    tc.tile_pool(name="ysb", bufs=4) as ypool,
                    tc.tile_pool(name="yps", bufs=8, space="PSUM") as ypsp,
                ):
                    ot_sb = []
                    for d in range(NDT):
                        otile = otpool.tile([128, TL], BF16, tag=f"ot{d}", name=f"ot{d}")
                        nc.sync.dma_start(
                            otile[:], a2a_out[d % 2][d // 2, :, :])
                        ot_sb.append(otile)
                        if _DEBUG and d == 0:
                            nc.sync.dma_start(dbg["ot0"][:], otile[:])
                    for tt in range(TL // 128):
                        yps = [ypsp.tile([128, 512], F32, tag="yps", name="ypsum")
                               for _ in range(4)]
                        for d in range(NDT):
                            for eo in range(4):
                                nc.tensor.matmul(
                                    yps[eo][:],
                                    ot_sb[d][:, tt * 128:(tt + 1) * 128],
                                    wo_sb[d][:, eo * 512:(eo + 1) * 512],
                                    start=(d == 0), stop=(d == NDT - 1))
                        for eo in range(4):
                            ysb = ypool.tile([128, 512], F32, tag="ysb", name="ysb")
                            nc.vector.tensor_copy(ysb[:], yps[eo][:])
                            if _DEBUG and tt == 0 and eo == 0:
                                nc.sync.dma_start(dbg["y0"][:], ysb[:])
                            nc.gpsimd.dma_start(
                                out[tt * 128:(tt + 1) * 128,
                                    eo * 512:(eo + 1) * 512], ysb[:])

    nc.compile()
    return nc


def _prep_in_maps(x, cos, sin, attn_mask, wq, wk, wv, wo):
    xf = np.ascontiguousarray(x.reshape(T, D).astype(np.float32, copy=False))
    cosT = np.ascontiguousarray(np.asarray(cos[0], np.float32).T)   # [HD, S]
    sinT = np.asarray(sin[0], np.float32).T
    sin_m = np.ascontiguousarray(
        np.concatenate([-sinT[:64], sinT[64:]], axis=0))            # [HD, S]
    mask_t = np.ascontiguousarray(
        np.asarray(attn_mask, np.float32).reshape(B * NKT, 128).T)  # [128, 32]
    wo_t = np.ascontiguousarray(np.asarray(wo, np.float32).T.astype(bf16))
    in_maps = []
    for i in range(NCORES):
        sl = slice(i * EL, (i + 1) * EL)
        in_maps.append({
            "x": xf,
            "wq_t": np.ascontiguousarray(np.asarray(wq, np.float32)[sl].T.astype(bf16)),
            "wk_t": np.ascontiguousarray(np.asarray(wk, np.float32)[sl].T.astype(bf16)),
            "wv_t": np.ascontiguousarray(np.asarray(wv, np.float32)[sl].T.astype(bf16)),
            "wo_t": wo_t,
            "cos_t": cosT,
            "sin_m": sin_m,
            "mask_t": mask_t,
        })
    return in_maps


def kernel(x, cos, sin, attn_mask, wq, wk, wv, wo, _trace=False):
    if "nc" not in _CACHE:
        _CACHE["nc"] = _build()
    nc = _CACHE["nc"]
    in_maps = _prep_in_maps(x, cos, sin, attn_mask, wq, wk, wv, wo)
    res = run_bass_kernel_spmd(nc, in_maps, core_ids=list(range(NCORES)),
                               trace=_trace)
    _CACHE["last_result"] = res
    y = np.concatenate([np.asarray(res.results[i]["out"], np.float32)
                        for i in range(NCORES)], axis=0)
    return y.reshape(B, S, D)
